# revision 1
# baseline (speedup 1.0000x reference)
"""MatchingNetwork forward on 8 Trainium2 NeuronCores.

The reference network's output reduces exactly to one_hot(labels, V) in f32:
the final einsum('btn,btv->btv', att, one_hot) sums att over n, and att is a
softmax over n, so the output is one_hot scaled by sum(softmax) == 1 (to float
rounding, ~1e-7).  Everything upstream (embedding gathers, BiLSTM GLayer,
attentional FLayer) cancels out of the result for every input.

So the kernel is a distributed one-hot materialization: B*T = 2048 rows of
V = 32000 f32 each (262 MB of output), data-parallel over rows across 8 cores
(256 rows/core = 32.77 MB/core of pure HBM writes -> memory-bound).

Per core: labels arrive as [128 partitions, 2] (row r = b*128 + p).  A single
SBUF iota row-chunk [128, CHUNK] is compared per (chunk, batch) against the
per-partition label via DVE tensor_scalar(subtract, is_equal), producing
1.0/0.0 f32 tiles that stream straight out to DRAM via HWDGE DMA.
"""

import os
import sys

for _p in ("/opt/trn_rl_repo", "/root/.axon_site/_ro/trn_rl_repo"):
    if os.path.isdir(_p) and _p not in sys.path:
        sys.path.append(_p)

import numpy as np

B, T, V = 32, 64, 32000
N_CORES = 8
ROWS = B * T                 # 2048 one-hot rows total
RPC = ROWS // N_CORES        # 256 rows per core
NB = RPC // 128              # 2 batches of 128 partitions
# Uniform column-chunk width.  2000 measured best: the 1 MB DMAs keep the
# write stream at ~415 GB/s while the first compare only waits on a ~2x1.8us
# gpsimd iota.  (8000 stalls startup ~20us; 1000 and mixed 500..4000 widths
# both measured slower in steady state.)
CHUNK = 2000
NCHUNK = V // CHUNK
MAXCHUNK = CHUNK
# iota built in two serial gpsimd pieces so scheduling can start DMAs while
# the second half generates.
IOTA_PIECES = [(0, 1000), (1000, 2000)]

_cache = {}


def _build_nc():
    import concourse.bacc as bacc
    import concourse.mybir as mybir
    from concourse.tile import TileContext

    nc = bacc.Bacc()
    lab_d = nc.dram_tensor("labels", [128, NB], mybir.dt.float32,
                           kind="ExternalInput")
    out_d = nc.dram_tensor("out", [NB, 128, V], mybir.dt.float32,
                           kind="ExternalOutput")

    with TileContext(nc) as tc:
        with tc.tile_pool(name="const", bufs=1) as cpool, \
             tc.tile_pool(name="work", bufs=6) as wpool:
            lab = cpool.tile([128, NB], mybir.dt.float32, tag="lab")
            nc.sync.dma_start(out=lab[:, :], in_=lab_d[:, :])
            iota = cpool.tile([128, MAXCHUNK], mybir.dt.float32, tag="iota")
            for (s, e) in IOTA_PIECES:
                nc.gpsimd.iota(iota[:, s:e], [[1, e - s]], base=s,
                               channel_multiplier=0,
                               allow_small_or_imprecise_dtypes=True)
            dma_engines = [nc.sync, nc.scalar]
            col = 0
            k = 0
            for w in [CHUNK] * NCHUNK:
                for b in range(NB):
                    o = wpool.tile([128, w], mybir.dt.float32, tag="o")
                    # o = is_equal(iota - (-col), lab[:, b])
                    #   = (global_col == label) ? 1.0 : 0.0
                    nc.vector.tensor_scalar(
                        out=o[:, :], in0=iota[:, :w],
                        scalar1=float(-col), scalar2=lab[:, b:b + 1],
                        op0=mybir.AluOpType.subtract,
                        op1=mybir.AluOpType.is_equal)
                    dma_engines[k % 2].dma_start(
                        out=out_d[b, :, col:col + w], in_=o[:, :])
                    k += 1
                col += w
    nc.finalize()
    return nc


def kernel(**inputs):
    from concourse.bass_utils import run_bass_kernel_spmd

    if "nc" not in _cache:
        _cache["nc"] = _build_nc()
    nc = _cache["nc"]

    # Label values < 2^24 are exact in f32.
    lab = np.asarray(inputs["labels"]).reshape(-1).astype(np.float32)
    in_maps = []
    for i in range(N_CORES):
        shard = lab[i * RPC:(i + 1) * RPC].reshape(NB, 128).T  # [128, NB]
        in_maps.append({"labels": np.ascontiguousarray(shard)})

    trace = bool(int(os.environ.get("BASS_KERNEL_TRACE", "0")))
    res = run_bass_kernel_spmd(nc, in_maps, list(range(N_CORES)), trace=trace)
    _cache["last_res"] = res

    outs = [res.results[i]["out"].reshape(RPC, V) for i in range(N_CORES)]
    return np.concatenate(outs, axis=0).reshape(B, T, V)

